# revision 16
# baseline (speedup 1.0000x reference)
"""Trainium2 Bass kernel for the BOW model (gather + segment-mean + MLP/BN/BCE).

Strategy (8 cores, data-parallel over segments):
  - Each core owns 512 contiguous segments (4 blocks x 128 segs).
  - Host planning is index-only: block boundaries via searchsorted, tokens
    sorted within each block (HBM locality) and split into 32768-row vocab
    quadrants so the int16-indexed dma_gather can address the table.
  - Device per core:
      * dma_gather (SWDGE ucode gather) of its tokens' embedding rows
      * one-hot masks built on DVE (iota + is_equal against seg_rel)
      * PE matmul one_hot^T @ gathered -> per-block segment sums (PSUM f32)
      * bow = sums * 1/max(counts,1); PE transpose -> bowT [H, segs]
      * zT = W1T^T @ bowT per block (features on partitions)
      * batch stats partial-reduced on free dim; AllReduce (4KB) across cores
      * BN + ReLU (per-partition scalars), logits = w2^T @ hT, + b2
      * partial BCE loss; AllReduce (4B); loss = sum/4096
  - Outputs: per-core logits slice [512] + loss [1]; host concatenates slices.
"""

import os
import numpy as np


def _ensure_import_path():
    try:
        import concourse.bass  # noqa: F401
        return
    except ImportError:
        pass
    import sys

    for p in ("/opt/trn_rl_repo", "/root/.axon_site/_ro/trn_rl_repo"):
        if os.path.isdir(p) and p not in sys.path:
            sys.path.insert(0, p)


_ensure_import_path()


def _install_ntff_hook_shim():
    """Provide antenv.axon_hooks (absent from the shipped antenv stub) so
    run_bass_kernel_spmd(trace=True) can capture NTFF profiles under axon."""
    try:
        from antenv.axon_hooks import get_axon_ntff_profile_hook  # noqa: F401
        return
    except ImportError:
        pass
    import sys
    import types
    import glob as _glob

    try:
        import antenv
    except ImportError:
        return
    mod = types.ModuleType("antenv.axon_hooks")
    state = {"hook": None}
    mod.set_axon_ntff_profile_hook = lambda h: state.__setitem__("hook", h)
    mod.get_axon_ntff_profile_hook = lambda: state["hook"]
    sys.modules["antenv.axon_hooks"] = mod
    antenv.axon_hooks = mod
    try:
        if "/root/.axon_site" not in sys.path:
            sys.path.append("/root/.axon_site")
        from trn_agent_boot.trn_boot import _ntff_profile_via_ctypes

        so_candidates = _glob.glob("/opt/axon/libaxon_pjrt*.so")
        if so_candidates:
            hook = _ntff_profile_via_ctypes(so_candidates[0])
            if hook is not None:
                mod.set_axon_ntff_profile_hook(hook)
    except Exception:
        pass


_install_ntff_hook_shim()

import concourse.mybir as mybir  # noqa: E402
import concourse.tile as tile  # noqa: E402
from concourse import bacc  # noqa: E402
from concourse.bass_utils import run_bass_kernel_spmd  # noqa: E402
from concourse.masks import make_identity  # noqa: E402

V, H, B, T = 100000, 512, 4096, 524288
EPS = 1e-5
NCORES = 8
SEGS_PER_CORE = B // NCORES  # 512
SEG_PER_BLOCK = 128
BLOCKS_PER_CORE = SEGS_PER_CORE // SEG_PER_BLOCK  # 4
JC = H // 128  # 4 feature chunks
HB = H // 128  # 4 h chunks
NCH_CALL = 8  # max gather chunks (of 128 rows) per dma_gather call
QUAD = 32768  # vocab quadrant size (int16 index limit)
NQUAD = (V + QUAD - 1) // QUAD  # 4

F32 = mybir.dt.float32
F32R = mybir.dt.float32r
I32 = mybir.dt.int32
I16 = mybir.dt.int16
AX = mybir.AxisListType
OP = mybir.AluOpType
ACTF = mybir.ActivationFunctionType

# segsum matmul dtype: fp32r (1 cyc/row) or fp32 (4 cyc/row, exact)
SEG_FP32R = bool(int(os.environ.get("SEG_FP32R", "1")))

LAST_RESULTS = None  # BassKernelResults of the most recent run (for test.py)

_IOTAM = np.ascontiguousarray(
    np.tile(np.arange(128, dtype=np.float32), (128, 1))
)

_program_cache = {}


def _build_program(run_chunks):
    """Build the SPMD bass program.

    run_chunks: tuple of 16 ints — chunk count for each (block lb, quadrant q)
    run, identical across cores (max over cores, data padded to match)."""
    run_chunks = np.asarray(run_chunks).reshape(BLOCKS_PER_CORE, NQUAD)
    nchunk = int(run_chunks.sum())
    SEG_DT = F32R if SEG_FP32R else F32

    nc = bacc.Bacc(
        "TRN2",
        target_bir_lowering=False,
        debug=False,
        num_devices=NCORES,
    )

    # ---- DRAM I/O ----
    emb_d = nc.dram_tensor("emb", [V, H], F32, kind="ExternalInput").ap()
    idx_d = nc.dram_tensor(
        "idx16", [128, nchunk * 8], I16, kind="ExternalInput"
    ).ap()
    seg_d = nc.dram_tensor("seg_rel", [128, nchunk], F32, kind="ExternalInput").ap()
    w1t_d = nc.dram_tensor("w1t", [H, H], F32, kind="ExternalInput").ap()
    gam_d = nc.dram_tensor("gamma", [H], F32, kind="ExternalInput").ap()
    bet_d = nc.dram_tensor("beta", [H], F32, kind="ExternalInput").ap()
    w2_d = nc.dram_tensor("w2", [H], F32, kind="ExternalInput").ap()
    t_d = nc.dram_tensor("t_loc", [SEGS_PER_CORE], F32, kind="ExternalInput").ap()
    b2_d = nc.dram_tensor("b2", [1], F32, kind="ExternalInput").ap()
    iota_d = nc.dram_tensor("iotam", [128, 128], F32, kind="ExternalInput").ap()
    invc_d = nc.dram_tensor(
        "inv_counts", [128, BLOCKS_PER_CORE], F32, kind="ExternalInput"
    ).ap()

    logits_d = nc.dram_tensor(
        "logits_out", [SEGS_PER_CORE], F32, kind="ExternalOutput"
    ).ap()
    loss_d = nc.dram_tensor("loss_out", [1], F32, kind="ExternalOutput").ap()

    rg = [list(range(NCORES))]

    with tile.TileContext(nc) as tc:
        with (
            tc.tile_pool(name="persist", bufs=1) as pp,
            tc.tile_pool(name="gath", bufs=6) as gp,
            tc.tile_pool(name="mask", bufs=4) as mp,
            tc.tile_pool(name="bowp", bufs=2) as bp,
            tc.tile_pool(name="small", bufs=2) as sp,
            tc.tile_pool(name="ps_acc", bufs=1, space="PSUM") as ps1,
            tc.tile_pool(name="ps_tr", bufs=2, space="PSUM") as ps2,
            tc.tile_pool(name="dram", bufs=1, space="DRAM") as dp,
        ):
            # ---- persistent SBUF state ----
            idx_sb = pp.tile([128, nchunk * 8], I16, tag="idx")
            seg_sb = pp.tile([128, nchunk], F32, tag="seg")
            w1t_sb = pp.tile([128, HB, H], F32, tag="w1t")
            gam_sb = pp.tile([128, JC], F32, tag="gam")
            bet_sb = pp.tile([128, JC], F32, tag="bet")
            w2_sb = pp.tile([128, JC], F32, tag="w2")
            t_sb = pp.tile([1, SEGS_PER_CORE], F32, tag="t")
            b2_sb = pp.tile([1, 1], F32, tag="b2")
            iota_sb = pp.tile([128, 128], F32, tag="iota")
            ident_sb = pp.tile([128, 128], F32, tag="ident")
            bowT_sb = pp.tile([128, HB, SEGS_PER_CORE], F32, tag="bowT")
            zT_sb = pp.tile([128, JC, SEGS_PER_CORE], F32, tag="zT")
            hT_sb = pp.tile([128, JC, SEGS_PER_CORE], F32, tag="hT")
            zsum_sb = pp.tile([128, JC, BLOCKS_PER_CORE], F32, tag="zsum")
            zsqsum_sb = pp.tile([128, JC, BLOCKS_PER_CORE], F32, tag="zsqsum")
            invc_sb = pp.tile([128, BLOCKS_PER_CORE], F32, tag="invc")
            stats_sb = pp.tile([128, 2 * JC], F32, tag="stats")
            statsg_sb = pp.tile([128, 2 * JC], F32, tag="statsg")

            nc.sync.dma_start(idx_sb[:], idx_d[:, :])
            nc.sync.dma_start(seg_sb[:], seg_d[:, :])
            nc.sync.dma_start(w1t_sb[:], w1t_d.rearrange("(hb p) j -> p hb j", p=128))
            nc.sync.dma_start(gam_sb[:], gam_d.rearrange("(c p) -> p c", p=128))
            nc.sync.dma_start(bet_sb[:], bet_d.rearrange("(c p) -> p c", p=128))
            nc.sync.dma_start(w2_sb[:], w2_d.rearrange("(c p) -> p c", p=128))
            nc.sync.dma_start(t_sb[:], t_d[None, :])
            nc.sync.dma_start(b2_sb[:], b2_d[None, :])
            nc.sync.dma_start(iota_sb[:], iota_d[:, :])
            nc.sync.dma_start(invc_sb[:], invc_d[:, :])
            make_identity(nc, ident_sb[:])

            # persistent PSUM accumulator
            sums_ps = ps1.tile([128, H], F32, tag="sums")

            def finish_block(lb):
                """Drain segment sums for block lb, build bowT slice, zT, stats."""
                seg_slice = slice(lb * 128, (lb + 1) * 128)
                bow_sb = bp.tile([128, H], F32, tag="bow")
                nc.vector.tensor_scalar(
                    bow_sb[:], sums_ps[:], invc_sb[:, lb : lb + 1], None, OP.mult
                )
                # transpose bow [seg, H] -> bowT slices [H, seg]
                for hb in range(HB):
                    tr_ps = ps2.tile([128, 128], F32, tag="tr")
                    nc.tensor.transpose(
                        tr_ps[:], bow_sb[:, hb * 128 : (hb + 1) * 128], ident_sb[:]
                    )
                    nc.vector.tensor_copy(bowT_sb[:, hb, seg_slice], tr_ps[:])
                # zT[jc, segs of lb] = sum_hb w1t[:, hb, jc]^T @ bowT[:, hb, segs]
                zt_ps = ps2.tile([128, JC, 128], F32, tag="zt")
                for jc in range(JC):
                    for hb in range(HB):
                        nc.tensor.matmul(
                            zt_ps[:, jc, :],
                            lhsT=w1t_sb[:, hb, jc * 128 : (jc + 1) * 128],
                            rhs=bowT_sb[:, hb, seg_slice],
                            start=(hb == 0),
                            stop=(hb == HB - 1),
                        )
                # drain zT block to SBUF
                nc.scalar.copy(zT_sb[:, :, seg_slice], zt_ps[:, :, :])
                # stats partials for this block
                zsq_sb = sp.tile([128, JC, 128], F32, tag="zsq")
                nc.scalar.square(zsq_sb[:], zt_ps[:, :, :])
                nc.vector.reduce_sum(
                    zsum_sb[:, :, lb : lb + 1], zT_sb[:, :, seg_slice], axis=AX.X
                )
                nc.vector.reduce_sum(
                    zsqsum_sb[:, :, lb : lb + 1], zsq_sb[:], axis=AX.X
                )

            # ---- main gather + segment-sum + per-block MLP feed ----
            # chunk layout: for lb in blocks, for q in quadrants: run_chunks[lb,q]
            # chunks. Gather calls stay within one (lb, q) run (single table
            # base). One matmul accumulation group per block.
            chunk0 = 0
            for lb in range(BLOCKS_PER_CORE):
                blk_nch = int(run_chunks[lb].sum())
                qoff = 0  # chunks of this block in earlier quadrant runs
                for q in range(NQUAD):
                    qn = int(run_chunks[lb, q])
                    qbase = q * QUAD
                    qsize = min(QUAD, V - qbase)
                    tabq = emb_d[qbase : qbase + qsize, :]
                    for cs in range(0, qn, NCH_CALL):
                        ncall = min(NCH_CALL, qn - cs)
                        col0 = chunk0 + qoff + cs
                        g_sb = gp.tile([128, NCH_CALL, H], F32, tag="g")
                        nc.gpsimd.dma_gather(
                            out_ap=g_sb[:, :ncall, :].bitcast(SEG_DT),
                            in_ap=tabq.bitcast(SEG_DT),
                            idxs_ap=idx_sb[:, col0 * 8 : (col0 + ncall) * 8],
                            num_idxs=ncall * 128,
                            num_idxs_reg=ncall * 128,
                            elem_size=H,
                            single_packet=False,
                        )
                        m_sb = mp.tile([128, NCH_CALL, 128], SEG_DT, tag="m")
                        for k in range(ncall):
                            j = col0 + k
                            kk = qoff + cs + k
                            nc.vector.tensor_tensor(
                                out=m_sb[:, k, :],
                                in0=iota_sb[:],
                                in1=seg_sb[:, j : j + 1].to_broadcast([128, 128]),
                                op=OP.is_equal,
                            )
                            lhsT = m_sb[:, k, :]
                            nc.tensor.matmul(
                                sums_ps[:],
                                lhsT=lhsT,
                                rhs=g_sb[:, k, :].bitcast(SEG_DT),
                                start=(kk == 0),
                                stop=(kk == blk_nch - 1),
                            )
                            if kk == blk_nch - 1:
                                finish_block(lb)
                    qoff += qn
                chunk0 += blk_nch

            # ---- final stats + AllReduce ----
            nc.vector.reduce_sum(stats_sb[:, 0:JC], zsum_sb[:], axis=AX.X)
            nc.vector.reduce_sum(stats_sb[:, JC : 2 * JC], zsqsum_sb[:], axis=AX.X)
            cc1_in = dp.tile([128, 2 * JC], F32, tag="cc1i")
            cc1_out = dp.tile([128, 2 * JC], F32, tag="cc1o")
            nc.sync.dma_start(cc1_in[:], stats_sb[:])
            nc.gpsimd.collective_compute(
                "AllReduce",
                OP.add,
                replica_groups=rg,
                ins=[cc1_in.opt()],
                outs=[cc1_out.opt()],
            )
            nc.sync.dma_start(statsg_sb[:], cc1_out[:])

            # ---- BN params ----
            mu_sb = pp.tile([128, JC], F32, tag="mu")
            var_sb = pp.tile([128, JC], F32, tag="var")
            tmp_sb = pp.tile([128, JC], F32, tag="tmpjc")
            std_sb = pp.tile([128, JC], F32, tag="std")
            rstd_sb = pp.tile([128, JC], F32, tag="rstd")
            a_sb = pp.tile([128, JC], F32, tag="abn")
            b_sb = pp.tile([128, JC], F32, tag="bbn")
            nc.vector.tensor_scalar_mul(mu_sb[:], statsg_sb[:, 0:JC], 1.0 / B)
            nc.vector.tensor_scalar_mul(var_sb[:], statsg_sb[:, JC : 2 * JC], 1.0 / B)
            nc.vector.tensor_tensor(tmp_sb[:], mu_sb[:], mu_sb[:], op=OP.mult)
            nc.vector.tensor_tensor(var_sb[:], var_sb[:], tmp_sb[:], op=OP.subtract)
            nc.vector.tensor_scalar_add(var_sb[:], var_sb[:], EPS)
            nc.scalar.activation(std_sb[:], var_sb[:], ACTF.Sqrt)
            nc.vector.reciprocal(rstd_sb[:], std_sb[:])
            nc.vector.tensor_tensor(a_sb[:], rstd_sb[:], gam_sb[:], op=OP.mult)
            nc.vector.tensor_tensor(tmp_sb[:], mu_sb[:], a_sb[:], op=OP.mult)
            nc.vector.tensor_tensor(b_sb[:], bet_sb[:], tmp_sb[:], op=OP.subtract)

            # ---- BN apply + relu + logits ----
            log_ps = ps1.tile([1, SEGS_PER_CORE], F32, tag="logps")
            for jc in range(JC):
                nc.vector.tensor_scalar(
                    hT_sb[:, jc, :],
                    zT_sb[:, jc, :],
                    a_sb[:, jc : jc + 1],
                    b_sb[:, jc : jc + 1],
                    OP.mult,
                    OP.add,
                )
                nc.scalar.activation(hT_sb[:, jc, :], hT_sb[:, jc, :], ACTF.Relu)
                nc.tensor.matmul(
                    log_ps[:],
                    lhsT=w2_sb[:, jc : jc + 1],
                    rhs=hT_sb[:, jc, :],
                    start=(jc == 0),
                    stop=(jc == JC - 1),
                )
            logits_sb = pp.tile([1, SEGS_PER_CORE], F32, tag="logits")
            nc.vector.tensor_scalar(
                logits_sb[:], log_ps[:], b2_sb[:, 0:1], None, OP.add
            )
            nc.sync.dma_start(logits_d[None, :], logits_sb[:])

            # ---- loss ----
            # softplus(x) = relu(x) + ln1p(exp(-|x|)); no Ln activation table
            # exists, so ln1p(u) is solved by Newton on e^z = 1+u:
            #   z <- z - 1 + (1+u) * exp(-z), 3 iterations from z0 = u.
            ax_sb = pp.tile([1, SEGS_PER_CORE], F32, tag="ax")
            u_sb = pp.tile([1, SEGS_PER_CORE], F32, tag="u")
            y_sb = pp.tile([1, SEGS_PER_CORE], F32, tag="y")
            z_sb = pp.tile([1, SEGS_PER_CORE], F32, tag="z")
            w_sb = pp.tile([1, SEGS_PER_CORE], F32, tag="w")
            sp_sb = pp.tile([1, SEGS_PER_CORE], F32, tag="sp")
            tl_sb = pp.tile([1, SEGS_PER_CORE], F32, tag="tl")
            ls_sb = pp.tile([1, 1], F32, tag="ls")
            lsg_sb = pp.tile([1, 1], F32, tag="lsg")
            nc.vector.tensor_scalar_mul(ax_sb[:], logits_sb[:], -1.0)
            nc.vector.tensor_tensor(ax_sb[:], logits_sb[:], ax_sb[:], op=OP.max)
            nc.scalar.activation(u_sb[:], ax_sb[:], ACTF.Exp, scale=-1.0)
            nc.vector.tensor_scalar_add(y_sb[:], u_sb[:], 1.0)
            nc.vector.tensor_copy(z_sb[:], u_sb[:])
            for _ in range(3):
                nc.scalar.activation(w_sb[:], z_sb[:], ACTF.Exp, scale=-1.0)
                nc.vector.tensor_tensor(w_sb[:], y_sb[:], w_sb[:], op=OP.mult)
                nc.vector.tensor_scalar_add(w_sb[:], w_sb[:], -1.0)
                nc.vector.tensor_tensor(z_sb[:], z_sb[:], w_sb[:], op=OP.add)
            nc.scalar.activation(sp_sb[:], logits_sb[:], ACTF.Relu)
            nc.vector.tensor_tensor(sp_sb[:], sp_sb[:], z_sb[:], op=OP.add)
            nc.vector.tensor_tensor(tl_sb[:], t_sb[:], logits_sb[:], op=OP.mult)
            nc.vector.tensor_tensor(tl_sb[:], sp_sb[:], tl_sb[:], op=OP.subtract)
            nc.vector.reduce_sum(ls_sb[:], tl_sb[:], axis=AX.X)
            cc2_in = dp.tile([1, 1], F32, tag="cc2i")
            cc2_out = dp.tile([1, 1], F32, tag="cc2o")
            nc.sync.dma_start(cc2_in[:], ls_sb[:])
            nc.gpsimd.collective_compute(
                "AllReduce",
                OP.add,
                replica_groups=rg,
                ins=[cc2_in.opt()],
                outs=[cc2_out.opt()],
            )
            nc.sync.dma_start(lsg_sb[:], cc2_out[:])
            nc.vector.tensor_scalar_mul(lsg_sb[:], lsg_sb[:], 1.0 / B)
            nc.sync.dma_start(loss_d[None, :], lsg_sb[:])

    nc.compile()
    return nc


def _plan_inputs(tokens, seg_ids, t, emb, W1, gamma, beta, w2, b2):
    """Index-only host planning: shard tokens by segment blocks, sort within
    blocks, split into vocab quadrants, pad to a shared chunk structure."""
    tokens = np.ascontiguousarray(np.asarray(tokens, dtype=np.int32))
    seg_ids = np.ascontiguousarray(np.asarray(seg_ids, dtype=np.int32))
    bounds = np.searchsorted(seg_ids, np.arange(0, B + 1, SEG_PER_BLOCK))

    # runs[c][lb][q] = (tok_local_sorted, seg_rel) for that vocab quadrant
    runs = [[[None] * NQUAD for _ in range(BLOCKS_PER_CORE)] for _ in range(NCORES)]
    for c in range(NCORES):
        for lb in range(BLOCKS_PER_CORE):
            g = c * BLOCKS_PER_CORE + lb
            s, e = bounds[g], bounds[g + 1]
            toks = tokens[s:e]
            segs = (seg_ids[s:e] - g * SEG_PER_BLOCK).astype(np.float32)
            order = np.argsort(toks, kind="stable")
            toks = toks[order]
            segs = segs[order]
            qsplit = np.searchsorted(toks, np.arange(1, NQUAD) * QUAD)
            tq = np.split(toks, qsplit)
            sq = np.split(segs, qsplit)
            for q in range(NQUAD):
                runs[c][lb][q] = (tq[q] - q * QUAD, sq[q])

    # shared chunk structure: per (lb, q) the max chunk count over cores
    run_chunks = np.zeros((BLOCKS_PER_CORE, NQUAD), np.int64)
    for lb in range(BLOCKS_PER_CORE):
        for q in range(NQUAD):
            mx = max(len(runs[c][lb][q][0]) for c in range(NCORES))
            run_chunks[lb, q] = (mx + 127) // 128
    nchunk = int(run_chunks.sum())

    w1t = np.ascontiguousarray(np.asarray(W1, dtype=np.float32).T)
    emb = np.ascontiguousarray(np.asarray(emb, dtype=np.float32))
    gamma = np.ascontiguousarray(np.asarray(gamma, dtype=np.float32))
    beta = np.ascontiguousarray(np.asarray(beta, dtype=np.float32))
    w2 = np.ascontiguousarray(np.asarray(w2, dtype=np.float32))
    t = np.asarray(t, dtype=np.float32)
    b2v = np.asarray(b2, dtype=np.float32).reshape(1)
    counts = np.bincount(seg_ids, minlength=B).astype(np.float32)
    inv_counts = 1.0 / np.maximum(counts, 1.0)

    in_maps = []
    for c in range(NCORES):
        idx_all = np.zeros((nchunk * 128,), np.int16)
        seg_all = np.full((nchunk * 128,), -1.0, np.float32)
        pos = 0
        for lb in range(BLOCKS_PER_CORE):
            for q in range(NQUAD):
                toks, segs = runs[c][lb][q]
                n = len(toks)
                cap = int(run_chunks[lb, q]) * 128
                idx_all[pos : pos + n] = toks.astype(np.int16)
                seg_all[pos : pos + n] = segs
                # padding: idx stays 0 (valid in-range row), seg_rel -1
                pos += cap
        # one-hot order: chunk j's token i sits at flat j*128+i; the mask
        # uses seg_sb[p, j] = seg_all[j*128+p]
        seg_pc = np.ascontiguousarray(seg_all.reshape(nchunk, 128).T)
        # dma_gather index wrap (per call): handled per NCH_CALL groups at
        # run granularity, matching the column slices used in the program.
        idx_cols = np.zeros((128, nchunk * 8), np.int16)
        chunk0 = 0
        for lb in range(BLOCKS_PER_CORE):
            for q in range(NQUAD):
                qn = int(run_chunks[lb, q])
                for cs in range(0, qn, NCH_CALL):
                    ncall = min(NCH_CALL, qn - cs)
                    col0 = chunk0 + cs
                    flat = idx_all[col0 * 128 : (col0 + ncall) * 128]
                    w = flat.reshape(-1, 16).T  # [16, ncall*8]
                    idx_cols[:, col0 * 8 : (col0 + ncall) * 8] = np.tile(w, (8, 1))
                chunk0 += qn
        in_maps.append(
            {
                "emb": emb,
                "idx16": np.ascontiguousarray(idx_cols),
                "seg_rel": seg_pc,
                "w1t": w1t,
                "gamma": gamma,
                "beta": beta,
                "w2": w2,
                "t_loc": np.ascontiguousarray(
                    t[c * SEGS_PER_CORE : (c + 1) * SEGS_PER_CORE]
                ),
                "b2": b2v,
                "iotam": _IOTAM,
                "inv_counts": np.ascontiguousarray(
                    inv_counts[c * SEGS_PER_CORE : (c + 1) * SEGS_PER_CORE]
                    .reshape(BLOCKS_PER_CORE, 128)
                    .T
                ),
            }
        )
    return tuple(run_chunks.ravel().tolist()), in_maps


def kernel(tokens, seg_ids, t, emb, W1, b1, gamma, beta, w2, b2):
    # b1 provably cancels inside BatchNorm (z+b1 - mean(z+b1) == z - mean(z),
    # var unchanged), so it is not shipped to the device.
    global LAST_RESULTS
    key, in_maps = _plan_inputs(
        tokens, seg_ids, t, emb, W1, gamma, beta, w2, b2
    )
    if key not in _program_cache:
        _program_cache[key] = _build_program(key)
    nc = _program_cache[key]

    trace = bool(os.environ.get("BASS_TRACE"))
    res = run_bass_kernel_spmd(
        nc,
        in_maps,
        core_ids=list(range(NCORES)),
        trace=trace,
        trace_cores=[0] if trace else None,
    )
    LAST_RESULTS = res
    logits = np.concatenate(
        [res.results[c]["logits_out"] for c in range(NCORES)]
    ).astype(np.float32)
    loss = np.asarray(res.results[0]["loss_out"][0], dtype=np.float32)
    return (loss, logits)


# revision 18
# speedup vs baseline: 1.0286x; 1.0286x over previous
"""Trainium2 Bass kernel for the BOW model (gather + segment-mean + MLP/BN/BCE).

Strategy (8 cores, data-parallel over segments):
  - Each core owns 512 contiguous segments (4 blocks x 128 segs).
  - Host planning is index-only: block boundaries via searchsorted, tokens
    sorted within each block (HBM locality) and split into 32768-row vocab
    quadrants so the int16-indexed dma_gather can address the table.
  - Device per core:
      * dma_gather (SWDGE ucode gather) of its tokens' embedding rows
      * one-hot masks built on DVE (iota + is_equal against seg_rel)
      * PE matmul one_hot^T @ gathered -> per-block segment sums (PSUM f32)
      * bow = sums * 1/max(counts,1); PE transpose -> bowT [H, segs]
      * zT = W1T^T @ bowT per block (features on partitions)
      * batch stats partial-reduced on free dim; AllReduce (4KB) across cores
      * BN + ReLU (per-partition scalars), logits = w2^T @ hT, + b2
      * partial BCE loss; AllReduce (4B); loss = sum/4096
  - Outputs: per-core logits slice [512] + loss [1]; host concatenates slices.
"""

import os
import numpy as np


def _ensure_import_path():
    try:
        import concourse.bass  # noqa: F401
        return
    except ImportError:
        pass
    import sys

    for p in ("/opt/trn_rl_repo", "/root/.axon_site/_ro/trn_rl_repo"):
        if os.path.isdir(p) and p not in sys.path:
            sys.path.insert(0, p)


_ensure_import_path()


def _install_ntff_hook_shim():
    """Provide antenv.axon_hooks (absent from the shipped antenv stub) so
    run_bass_kernel_spmd(trace=True) can capture NTFF profiles under axon."""
    try:
        from antenv.axon_hooks import get_axon_ntff_profile_hook  # noqa: F401
        return
    except ImportError:
        pass
    import sys
    import types
    import glob as _glob

    try:
        import antenv
    except ImportError:
        return
    mod = types.ModuleType("antenv.axon_hooks")
    state = {"hook": None}
    mod.set_axon_ntff_profile_hook = lambda h: state.__setitem__("hook", h)
    mod.get_axon_ntff_profile_hook = lambda: state["hook"]
    sys.modules["antenv.axon_hooks"] = mod
    antenv.axon_hooks = mod
    try:
        if "/root/.axon_site" not in sys.path:
            sys.path.append("/root/.axon_site")
        from trn_agent_boot.trn_boot import _ntff_profile_via_ctypes

        so_candidates = _glob.glob("/opt/axon/libaxon_pjrt*.so")
        if so_candidates:
            hook = _ntff_profile_via_ctypes(so_candidates[0])
            if hook is not None:
                mod.set_axon_ntff_profile_hook(hook)
    except Exception:
        pass


_install_ntff_hook_shim()

import concourse.mybir as mybir  # noqa: E402
import concourse.tile as tile  # noqa: E402
from concourse import bacc  # noqa: E402
from concourse.bass_utils import run_bass_kernel_spmd  # noqa: E402
from concourse.masks import make_identity  # noqa: E402

V, H, B, T = 100000, 512, 4096, 524288
EPS = 1e-5
NCORES = 8
SEGS_PER_CORE = B // NCORES  # 512
SEG_PER_BLOCK = 128
BLOCKS_PER_CORE = SEGS_PER_CORE // SEG_PER_BLOCK  # 4
JC = H // 128  # 4 feature chunks
HB = H // 128  # 4 h chunks
NCH_CALL = 12  # max gather chunks (of 128 rows) per dma_gather call
QUAD = 32768  # vocab quadrant size (int16 index limit)
NQUAD = (V + QUAD - 1) // QUAD  # 4

F32 = mybir.dt.float32
F32R = mybir.dt.float32r
I32 = mybir.dt.int32
I16 = mybir.dt.int16
AX = mybir.AxisListType
OP = mybir.AluOpType
ACTF = mybir.ActivationFunctionType

# segsum matmul dtype: fp32r (1 cyc/row) or fp32 (4 cyc/row, exact)
SEG_FP32R = bool(int(os.environ.get("SEG_FP32R", "1")))

LAST_RESULTS = None  # BassKernelResults of the most recent run (for test.py)

_IOTAM = np.ascontiguousarray(
    np.tile(np.arange(128, dtype=np.float32), (128, 1))
)

_program_cache = {}


def _build_program(run_chunks):
    """Build the SPMD bass program.

    run_chunks: tuple of 16 ints — chunk count for each (block lb, quadrant q)
    run, identical across cores (max over cores, data padded to match)."""
    run_chunks = np.asarray(run_chunks).reshape(BLOCKS_PER_CORE, NQUAD)
    nchunk = int(run_chunks.sum())
    SEG_DT = F32R if SEG_FP32R else F32

    nc = bacc.Bacc(
        "TRN2",
        target_bir_lowering=False,
        debug=False,
        num_devices=NCORES,
    )

    # ---- DRAM I/O ----
    emb_d = nc.dram_tensor("emb", [V, H], F32, kind="ExternalInput").ap()
    idx_d = nc.dram_tensor(
        "idx16", [128, nchunk * 8], I16, kind="ExternalInput"
    ).ap()
    seg_d = nc.dram_tensor("seg_rel", [128, nchunk], F32, kind="ExternalInput").ap()
    w1t_d = nc.dram_tensor("w1t", [H, H], F32, kind="ExternalInput").ap()
    gam_d = nc.dram_tensor("gamma", [H], F32, kind="ExternalInput").ap()
    bet_d = nc.dram_tensor("beta", [H], F32, kind="ExternalInput").ap()
    w2_d = nc.dram_tensor("w2", [H], F32, kind="ExternalInput").ap()
    t_d = nc.dram_tensor("t_loc", [SEGS_PER_CORE], F32, kind="ExternalInput").ap()
    b2_d = nc.dram_tensor("b2", [1], F32, kind="ExternalInput").ap()
    iota_d = nc.dram_tensor("iotam", [128, 128], F32, kind="ExternalInput").ap()
    invc_d = nc.dram_tensor(
        "inv_counts", [128, BLOCKS_PER_CORE], F32, kind="ExternalInput"
    ).ap()

    logits_d = nc.dram_tensor(
        "logits_out", [SEGS_PER_CORE], F32, kind="ExternalOutput"
    ).ap()
    loss_d = nc.dram_tensor("loss_out", [1], F32, kind="ExternalOutput").ap()

    rg = [list(range(NCORES))]

    with tile.TileContext(nc) as tc:
        with (
            tc.tile_pool(name="persist", bufs=1) as pp,
            tc.tile_pool(name="gath", bufs=4) as gp,
            tc.tile_pool(name="mask", bufs=3) as mp,
            tc.tile_pool(name="bowp", bufs=2) as bp,
            tc.tile_pool(name="small", bufs=2) as sp,
            tc.tile_pool(name="ps_acc", bufs=1, space="PSUM") as ps1,
            tc.tile_pool(name="ps_tr", bufs=2, space="PSUM") as ps2,
            tc.tile_pool(name="dram", bufs=1, space="DRAM") as dp,
        ):
            # ---- persistent SBUF state ----
            idx_sb = pp.tile([128, nchunk * 8], I16, tag="idx")
            seg_sb = pp.tile([128, nchunk], F32, tag="seg")
            w1t_sb = pp.tile([128, HB, H], F32, tag="w1t")
            gam_sb = pp.tile([128, JC], F32, tag="gam")
            bet_sb = pp.tile([128, JC], F32, tag="bet")
            w2_sb = pp.tile([128, JC], F32, tag="w2")
            t_sb = pp.tile([1, SEGS_PER_CORE], F32, tag="t")
            b2_sb = pp.tile([1, 1], F32, tag="b2")
            iota_sb = pp.tile([128, 128], F32, tag="iota")
            ident_sb = pp.tile([128, 128], F32, tag="ident")
            bowT_sb = pp.tile([128, HB, SEGS_PER_CORE], F32, tag="bowT")
            zT_sb = pp.tile([128, JC, SEGS_PER_CORE], F32, tag="zT")
            hT_sb = pp.tile([128, JC, SEGS_PER_CORE], F32, tag="hT")
            zsum_sb = pp.tile([128, JC, BLOCKS_PER_CORE], F32, tag="zsum")
            zsqsum_sb = pp.tile([128, JC, BLOCKS_PER_CORE], F32, tag="zsqsum")
            invc_sb = pp.tile([128, BLOCKS_PER_CORE], F32, tag="invc")
            stats_sb = pp.tile([128, 2 * JC], F32, tag="stats")
            statsg_sb = pp.tile([128, 2 * JC], F32, tag="statsg")

            nc.sync.dma_start(idx_sb[:], idx_d[:, :])
            nc.sync.dma_start(seg_sb[:], seg_d[:, :])
            nc.sync.dma_start(w1t_sb[:], w1t_d.rearrange("(hb p) j -> p hb j", p=128))
            nc.sync.dma_start(gam_sb[:], gam_d.rearrange("(c p) -> p c", p=128))
            nc.sync.dma_start(bet_sb[:], bet_d.rearrange("(c p) -> p c", p=128))
            nc.sync.dma_start(w2_sb[:], w2_d.rearrange("(c p) -> p c", p=128))
            nc.sync.dma_start(t_sb[:], t_d[None, :])
            nc.sync.dma_start(b2_sb[:], b2_d[None, :])
            nc.sync.dma_start(iota_sb[:], iota_d[:, :])
            nc.sync.dma_start(invc_sb[:], invc_d[:, :])
            make_identity(nc, ident_sb[:])

            # persistent PSUM accumulator
            sums_ps = ps1.tile([128, H], F32, tag="sums")

            def finish_block(lb):
                """Drain segment sums for block lb, build bowT slice, zT, stats."""
                seg_slice = slice(lb * 128, (lb + 1) * 128)
                bow_sb = bp.tile([128, H], F32, tag="bow")
                nc.vector.tensor_scalar(
                    bow_sb[:], sums_ps[:], invc_sb[:, lb : lb + 1], None, OP.mult
                )
                # transpose bow [seg, H] -> bowT slices [H, seg]
                for hb in range(HB):
                    tr_ps = ps2.tile([128, 128], F32, tag="tr")
                    nc.tensor.transpose(
                        tr_ps[:], bow_sb[:, hb * 128 : (hb + 1) * 128], ident_sb[:]
                    )
                    nc.vector.tensor_copy(bowT_sb[:, hb, seg_slice], tr_ps[:])
                # zT[jc, segs of lb] = sum_hb w1t[:, hb, jc]^T @ bowT[:, hb, segs]
                zt_ps = ps2.tile([128, JC, 128], F32, tag="zt")
                for jc in range(JC):
                    for hb in range(HB):
                        nc.tensor.matmul(
                            zt_ps[:, jc, :],
                            lhsT=w1t_sb[:, hb, jc * 128 : (jc + 1) * 128],
                            rhs=bowT_sb[:, hb, seg_slice],
                            start=(hb == 0),
                            stop=(hb == HB - 1),
                        )
                # drain zT block to SBUF
                nc.scalar.copy(zT_sb[:, :, seg_slice], zt_ps[:, :, :])
                # stats partials for this block
                zsq_sb = sp.tile([128, JC, 128], F32, tag="zsq")
                nc.scalar.square(zsq_sb[:], zt_ps[:, :, :])
                nc.vector.reduce_sum(
                    zsum_sb[:, :, lb : lb + 1], zT_sb[:, :, seg_slice], axis=AX.X
                )
                nc.vector.reduce_sum(
                    zsqsum_sb[:, :, lb : lb + 1], zsq_sb[:], axis=AX.X
                )

            # ---- main gather + segment-sum + per-block MLP feed ----
            # chunk layout: for lb in blocks, for q in quadrants: run_chunks[lb,q]
            # chunks. Gather calls stay within one (lb, q) run (single table
            # base). One matmul accumulation group per block.
            chunk0 = 0
            for lb in range(BLOCKS_PER_CORE):
                blk_nch = int(run_chunks[lb].sum())
                qoff = 0  # chunks of this block in earlier quadrant runs
                for q in range(NQUAD):
                    qn = int(run_chunks[lb, q])
                    qbase = q * QUAD
                    qsize = min(QUAD, V - qbase)
                    tabq = emb_d[qbase : qbase + qsize, :]
                    for cs in range(0, qn, NCH_CALL):
                        ncall = min(NCH_CALL, qn - cs)
                        col0 = chunk0 + qoff + cs
                        g_sb = gp.tile([128, NCH_CALL, H], F32, tag="g")
                        nc.gpsimd.dma_gather(
                            out_ap=g_sb[:, :ncall, :].bitcast(SEG_DT),
                            in_ap=tabq.bitcast(SEG_DT),
                            idxs_ap=idx_sb[:, col0 * 8 : (col0 + ncall) * 8],
                            num_idxs=ncall * 128,
                            num_idxs_reg=ncall * 128,
                            elem_size=H,
                            single_packet=False,
                        )
                        m_sb = mp.tile([128, NCH_CALL, 128], SEG_DT, tag="m")
                        for k in range(ncall):
                            j = col0 + k
                            kk = qoff + cs + k
                            nc.vector.tensor_tensor(
                                out=m_sb[:, k, :],
                                in0=iota_sb[:],
                                in1=seg_sb[:, j : j + 1].to_broadcast([128, 128]),
                                op=OP.is_equal,
                            )
                            lhsT = m_sb[:, k, :]
                            nc.tensor.matmul(
                                sums_ps[:],
                                lhsT=lhsT,
                                rhs=g_sb[:, k, :].bitcast(SEG_DT),
                                start=(kk == 0),
                                stop=(kk == blk_nch - 1),
                            )
                            if kk == blk_nch - 1:
                                finish_block(lb)
                    qoff += qn
                chunk0 += blk_nch

            # ---- final stats + AllReduce ----
            nc.vector.reduce_sum(stats_sb[:, 0:JC], zsum_sb[:], axis=AX.X)
            nc.vector.reduce_sum(stats_sb[:, JC : 2 * JC], zsqsum_sb[:], axis=AX.X)
            cc1_in = dp.tile([128, 2 * JC], F32, tag="cc1i")
            cc1_out = dp.tile([128, 2 * JC], F32, tag="cc1o")
            nc.sync.dma_start(cc1_in[:], stats_sb[:])
            nc.gpsimd.collective_compute(
                "AllReduce",
                OP.add,
                replica_groups=rg,
                ins=[cc1_in.opt()],
                outs=[cc1_out.opt()],
            )
            nc.sync.dma_start(statsg_sb[:], cc1_out[:])

            # ---- BN params ----
            mu_sb = pp.tile([128, JC], F32, tag="mu")
            var_sb = pp.tile([128, JC], F32, tag="var")
            tmp_sb = pp.tile([128, JC], F32, tag="tmpjc")
            std_sb = pp.tile([128, JC], F32, tag="std")
            rstd_sb = pp.tile([128, JC], F32, tag="rstd")
            a_sb = pp.tile([128, JC], F32, tag="abn")
            b_sb = pp.tile([128, JC], F32, tag="bbn")
            nc.vector.tensor_scalar_mul(mu_sb[:], statsg_sb[:, 0:JC], 1.0 / B)
            nc.vector.tensor_scalar_mul(var_sb[:], statsg_sb[:, JC : 2 * JC], 1.0 / B)
            nc.vector.tensor_tensor(tmp_sb[:], mu_sb[:], mu_sb[:], op=OP.mult)
            nc.vector.tensor_tensor(var_sb[:], var_sb[:], tmp_sb[:], op=OP.subtract)
            nc.vector.tensor_scalar_add(var_sb[:], var_sb[:], EPS)
            nc.scalar.activation(std_sb[:], var_sb[:], ACTF.Sqrt)
            nc.vector.reciprocal(rstd_sb[:], std_sb[:])
            nc.vector.tensor_tensor(a_sb[:], rstd_sb[:], gam_sb[:], op=OP.mult)
            nc.vector.tensor_tensor(tmp_sb[:], mu_sb[:], a_sb[:], op=OP.mult)
            nc.vector.tensor_tensor(b_sb[:], bet_sb[:], tmp_sb[:], op=OP.subtract)

            # ---- BN apply + relu + logits ----
            log_ps = ps1.tile([1, SEGS_PER_CORE], F32, tag="logps")
            for jc in range(JC):
                nc.vector.tensor_scalar(
                    hT_sb[:, jc, :],
                    zT_sb[:, jc, :],
                    a_sb[:, jc : jc + 1],
                    b_sb[:, jc : jc + 1],
                    OP.mult,
                    OP.add,
                )
                nc.scalar.activation(hT_sb[:, jc, :], hT_sb[:, jc, :], ACTF.Relu)
                nc.tensor.matmul(
                    log_ps[:],
                    lhsT=w2_sb[:, jc : jc + 1],
                    rhs=hT_sb[:, jc, :],
                    start=(jc == 0),
                    stop=(jc == JC - 1),
                )
            logits_sb = pp.tile([1, SEGS_PER_CORE], F32, tag="logits")
            nc.vector.tensor_scalar(
                logits_sb[:], log_ps[:], b2_sb[:, 0:1], None, OP.add
            )
            nc.sync.dma_start(logits_d[None, :], logits_sb[:])

            # ---- loss ----
            # softplus(x) = relu(x) + ln1p(exp(-|x|)); no Ln activation table
            # exists, so ln1p(u) is solved by Newton on e^z = 1+u:
            #   z <- z - 1 + (1+u) * exp(-z), 3 iterations from z0 = u.
            ax_sb = pp.tile([1, SEGS_PER_CORE], F32, tag="ax")
            u_sb = pp.tile([1, SEGS_PER_CORE], F32, tag="u")
            y_sb = pp.tile([1, SEGS_PER_CORE], F32, tag="y")
            z_sb = pp.tile([1, SEGS_PER_CORE], F32, tag="z")
            w_sb = pp.tile([1, SEGS_PER_CORE], F32, tag="w")
            sp_sb = pp.tile([1, SEGS_PER_CORE], F32, tag="sp")
            tl_sb = pp.tile([1, SEGS_PER_CORE], F32, tag="tl")
            ls_sb = pp.tile([1, 1], F32, tag="ls")
            lsg_sb = pp.tile([1, 1], F32, tag="lsg")
            nc.vector.tensor_scalar_mul(ax_sb[:], logits_sb[:], -1.0)
            nc.vector.tensor_tensor(ax_sb[:], logits_sb[:], ax_sb[:], op=OP.max)
            nc.scalar.activation(u_sb[:], ax_sb[:], ACTF.Exp, scale=-1.0)
            nc.vector.tensor_scalar_add(y_sb[:], u_sb[:], 1.0)
            nc.vector.tensor_copy(z_sb[:], u_sb[:])
            for _ in range(3):
                nc.scalar.activation(w_sb[:], z_sb[:], ACTF.Exp, scale=-1.0)
                nc.vector.tensor_tensor(w_sb[:], y_sb[:], w_sb[:], op=OP.mult)
                nc.vector.tensor_scalar_add(w_sb[:], w_sb[:], -1.0)
                nc.vector.tensor_tensor(z_sb[:], z_sb[:], w_sb[:], op=OP.add)
            nc.scalar.activation(sp_sb[:], logits_sb[:], ACTF.Relu)
            nc.vector.tensor_tensor(sp_sb[:], sp_sb[:], z_sb[:], op=OP.add)
            nc.vector.tensor_tensor(tl_sb[:], t_sb[:], logits_sb[:], op=OP.mult)
            nc.vector.tensor_tensor(tl_sb[:], sp_sb[:], tl_sb[:], op=OP.subtract)
            nc.vector.reduce_sum(ls_sb[:], tl_sb[:], axis=AX.X)
            cc2_in = dp.tile([1, 1], F32, tag="cc2i")
            cc2_out = dp.tile([1, 1], F32, tag="cc2o")
            nc.sync.dma_start(cc2_in[:], ls_sb[:])
            nc.gpsimd.collective_compute(
                "AllReduce",
                OP.add,
                replica_groups=rg,
                ins=[cc2_in.opt()],
                outs=[cc2_out.opt()],
            )
            nc.sync.dma_start(lsg_sb[:], cc2_out[:])
            nc.vector.tensor_scalar_mul(lsg_sb[:], lsg_sb[:], 1.0 / B)
            nc.sync.dma_start(loss_d[None, :], lsg_sb[:])

    nc.compile()
    return nc


def _plan_inputs(tokens, seg_ids, t, emb, W1, gamma, beta, w2, b2):
    """Index-only host planning: shard tokens by segment blocks, sort within
    blocks, split into vocab quadrants, pad to a shared chunk structure."""
    tokens = np.ascontiguousarray(np.asarray(tokens, dtype=np.int32))
    seg_ids = np.ascontiguousarray(np.asarray(seg_ids, dtype=np.int32))
    bounds = np.searchsorted(seg_ids, np.arange(0, B + 1, SEG_PER_BLOCK))

    # runs[c][lb][q] = (tok_local_sorted, seg_rel) for that vocab quadrant
    runs = [[[None] * NQUAD for _ in range(BLOCKS_PER_CORE)] for _ in range(NCORES)]
    for c in range(NCORES):
        for lb in range(BLOCKS_PER_CORE):
            g = c * BLOCKS_PER_CORE + lb
            s, e = bounds[g], bounds[g + 1]
            toks = tokens[s:e]
            segs = (seg_ids[s:e] - g * SEG_PER_BLOCK).astype(np.float32)
            order = np.argsort(toks, kind="stable")
            toks = toks[order]
            segs = segs[order]
            qsplit = np.searchsorted(toks, np.arange(1, NQUAD) * QUAD)
            tq = np.split(toks, qsplit)
            sq = np.split(segs, qsplit)
            for q in range(NQUAD):
                runs[c][lb][q] = (tq[q] - q * QUAD, sq[q])

    # shared chunk structure: per (lb, q) the max chunk count over cores
    run_chunks = np.zeros((BLOCKS_PER_CORE, NQUAD), np.int64)
    for lb in range(BLOCKS_PER_CORE):
        for q in range(NQUAD):
            mx = max(len(runs[c][lb][q][0]) for c in range(NCORES))
            run_chunks[lb, q] = (mx + 127) // 128
    nchunk = int(run_chunks.sum())

    w1t = np.ascontiguousarray(np.asarray(W1, dtype=np.float32).T)
    emb = np.ascontiguousarray(np.asarray(emb, dtype=np.float32))
    gamma = np.ascontiguousarray(np.asarray(gamma, dtype=np.float32))
    beta = np.ascontiguousarray(np.asarray(beta, dtype=np.float32))
    w2 = np.ascontiguousarray(np.asarray(w2, dtype=np.float32))
    t = np.asarray(t, dtype=np.float32)
    b2v = np.asarray(b2, dtype=np.float32).reshape(1)
    counts = np.bincount(seg_ids, minlength=B).astype(np.float32)
    inv_counts = 1.0 / np.maximum(counts, 1.0)

    in_maps = []
    for c in range(NCORES):
        idx_all = np.zeros((nchunk * 128,), np.int16)
        seg_all = np.full((nchunk * 128,), -1.0, np.float32)
        pos = 0
        for lb in range(BLOCKS_PER_CORE):
            for q in range(NQUAD):
                toks, segs = runs[c][lb][q]
                n = len(toks)
                cap = int(run_chunks[lb, q]) * 128
                idx_all[pos : pos + n] = toks.astype(np.int16)
                seg_all[pos : pos + n] = segs
                if n and n < cap:
                    # pad with the run's last row: keeps the gather stream
                    # monotone (seg_rel stays -1, so contributions are masked)
                    idx_all[pos + n : pos + cap] = np.int16(toks[-1])
                # padding: idx stays 0 (valid in-range row), seg_rel -1
                pos += cap
        # one-hot order: chunk j's token i sits at flat j*128+i; the mask
        # uses seg_sb[p, j] = seg_all[j*128+p]
        seg_pc = np.ascontiguousarray(seg_all.reshape(nchunk, 128).T)
        # dma_gather index wrap (per call): handled per NCH_CALL groups at
        # run granularity, matching the column slices used in the program.
        idx_cols = np.zeros((128, nchunk * 8), np.int16)
        chunk0 = 0
        for lb in range(BLOCKS_PER_CORE):
            for q in range(NQUAD):
                qn = int(run_chunks[lb, q])
                for cs in range(0, qn, NCH_CALL):
                    ncall = min(NCH_CALL, qn - cs)
                    col0 = chunk0 + cs
                    flat = idx_all[col0 * 128 : (col0 + ncall) * 128]
                    w = flat.reshape(-1, 16).T  # [16, ncall*8]
                    idx_cols[:, col0 * 8 : (col0 + ncall) * 8] = np.tile(w, (8, 1))
                chunk0 += qn
        in_maps.append(
            {
                "emb": emb,
                "idx16": np.ascontiguousarray(idx_cols),
                "seg_rel": seg_pc,
                "w1t": w1t,
                "gamma": gamma,
                "beta": beta,
                "w2": w2,
                "t_loc": np.ascontiguousarray(
                    t[c * SEGS_PER_CORE : (c + 1) * SEGS_PER_CORE]
                ),
                "b2": b2v,
                "iotam": _IOTAM,
                "inv_counts": np.ascontiguousarray(
                    inv_counts[c * SEGS_PER_CORE : (c + 1) * SEGS_PER_CORE]
                    .reshape(BLOCKS_PER_CORE, 128)
                    .T
                ),
            }
        )
    return tuple(run_chunks.ravel().tolist()), in_maps


def kernel(tokens, seg_ids, t, emb, W1, b1, gamma, beta, w2, b2):
    # b1 provably cancels inside BatchNorm (z+b1 - mean(z+b1) == z - mean(z),
    # var unchanged), so it is not shipped to the device.
    global LAST_RESULTS
    key, in_maps = _plan_inputs(
        tokens, seg_ids, t, emb, W1, gamma, beta, w2, b2
    )
    if key not in _program_cache:
        _program_cache[key] = _build_program(key)
    nc = _program_cache[key]

    trace = bool(os.environ.get("BASS_TRACE"))
    res = run_bass_kernel_spmd(
        nc,
        in_maps,
        core_ids=list(range(NCORES)),
        trace=trace,
        trace_cores=[0] if trace else None,
    )
    LAST_RESULTS = res
    logits = np.concatenate(
        [res.results[c]["logits_out"] for c in range(NCORES)]
    ).astype(np.float32)
    loss = np.asarray(res.results[0]["loss_out"][0], dtype=np.float32)
    return (loss, logits)
